# revision 62
# baseline (speedup 1.0000x reference)
"""Trainium2 Bass kernel for nn_DeconvLayer (cascaded order-16 IIR along rows).

Math: reference applies a causal order-16 linear recurrence with taps h
(then again with reversed taps) along each row of a [4096, 4096] f32 matrix,
with the first K=16 outputs forced to zero and x[i] entering only for i >= K.

This is equivalent to  y = g (*) x_masked  where x_masked zeroes columns
0..15 and g is the (rapidly decaying) impulse response of the cascaded
filter. |g[t]| < 4e-7 for t >= 129 for stable taps (|h| ~ 0.05), so a
truncated FIR of ~256 taps is exact to f32 precision.

On-device layout: the per-row convolution contracts along time, so time must
sit on SBUF partitions. Rows are sharded 512/core across 8 cores. Per core:
  - plain fp16 loads (rows on partitions), TensorE 128x128 transposes build
    U_b[t, r] = x[r, 128*b + t] in PSUM, DVE copies them to SBUF
  - TensorE computes out[r, 128*ot + m] = sum_d sum_k U_{ot-d}[k, r] * G_d[k, m]
    with G_d[k, m] = g[128*d + m - k]  (Toeplitz slabs, D=2), f32 PSUM accum
  - DVE/ACT cast-copy PSUM->SBUF fp16, DMA out fp16 (host upcasts to f32)
"""

import os
import time

import numpy as np

# the trace path needs antenv.axon_hooks, absent in this container; make
# sure a stray BASS_TRACE in the caller's env can't break execution
os.environ.setdefault("BASS_NEVER_TRACE", "1")

import concourse.bass as bass
import concourse.mybir as mybir
from concourse.bass_utils import run_bass_kernel_spmd
from concourse.tile import TileContext

N_CORES = 8
ROWS = 4096
COLS = 4096
ROWS_PER_CORE = ROWS // N_CORES  # 512
K_TAPS = 16
D = 2          # matmul depth: output tile ot reads input tiles ot, ot-1
T_FIR = 256    # taps kept when building G (effective coverage per output m: m+129)
NT = COLS // 128   # 32 time tiles
NQ = NT // 4       # 8 column panels of 512
NRC = ROWS_PER_CORE // 128  # 4 row chunks per core

_F16 = mybir.dt.float16
_F32 = mybir.dt.float32


def _impulse_response(h: np.ndarray, n: int) -> np.ndarray:
    """Impulse response of v[i] = x[i] + sum_j h[j] v[i-1-j], float64."""
    g = np.zeros(n, np.float64)
    g[0] = 1.0
    K = len(h)
    for t in range(1, n):
        lo = max(0, t - K)
        g[t] = np.dot(h[: t - lo], g[t - 1 : lo - 1 if lo > 0 else None : -1])
    return g


def _build_g_cat(h32: np.ndarray) -> np.ndarray:
    """[128, 128*D] fp16: G_d[k, m] = g[128*d + m - k] columns concatenated."""
    h = h32.astype(np.float64)
    g1 = _impulse_response(h, T_FIR)
    g2 = _impulse_response(h[::-1], T_FIR)
    gc = np.convolve(g1, g2)[:T_FIR]
    kk = np.arange(128)[:, None]
    mm = np.arange(128)[None, :]
    mats = []
    for d in range(D):
        t = 128 * d + mm - kk
        valid = (t >= 0) & (t < T_FIR)
        mats.append(np.where(valid, gc[np.clip(t, 0, T_FIR - 1)], 0.0))
    return np.concatenate(mats, axis=1).astype(np.float16)


def _build_program(legalize: bool = True) -> bass.Bass:
    """Per-core program.

    Engine roles (balanced so each engine's busy time is ~16-21us and the
    schedule is PE-bound; _legalize_waits post-pass keeps every instruction
    within this walrus' one-semaphore-wait capacity):
      - GPSIMD (SWDGE): x loads (pieces, for early pipeline fill)
      - PE: 128x128 fp16 transposes + Toeplitz conv matmuls (f32 PSUM)
      - DVE: transposed-tile PSUM->SBUF copies (uint32-bitcast) + 1/4 of
        the conv-result cast-copies
      - ACT: 3/4 of the conv-result f32->fp16 cast-copies
      - SP (HWDGE): const loads + stores (last row-chunk's stores split
        with Pool to parallelize the tail)
    """
    nc = bass.Bass()
    x = nc.dram_tensor("x", [ROWS_PER_CORE, COLS], _F16, kind="ExternalInput")
    g = nc.dram_tensor("g", [128, 128 * D], _F16, kind="ExternalInput")
    ident = nc.dram_tensor("ident", [128, 128], _F16, kind="ExternalInput")
    # fp16 output: halves store bytes (the HBM floor dominates e2e); costs
    # ~1e-4 extra rounding (host upcasts to f32)
    y = nc.dram_tensor("y", [ROWS_PER_CORE, COLS], _F16, kind="ExternalOutput")

    with TileContext(nc) as tc:
        with (
            tc.tile_pool(name="cpool", bufs=1) as cpool,
            tc.tile_pool(name="xpool", bufs=1) as xpool,
            tc.tile_pool(name="upool", bufs=2) as upool,
            tc.tile_pool(name="ptpool", bufs=3, space="PSUM") as ptpool,
            tc.tile_pool(name="popool", bufs=5, space="PSUM") as popool,
            tc.tile_pool(name="ypool", bufs=3) as ypool,
        ):
            # ident first on SP — it gates the first transposes; g is loaded
            # after rc0's x pieces (only needed once conv starts)
            idt = cpool.tile([128, 128], _F16, tag="id")
            nc.sync.dma_start(idt[:], ident[:])
            gt = cpool.tile([128, 128 * D], _F16, tag="g")

            for rc in range(4):
                rs = slice(128 * rc, 128 * (rc + 1))
                # x loaded in pieces so early transposes start sooner; the
                # first row-chunk uses eighth-loads for fast pipeline fill
                npieces = 8 if rc == 0 else 2
                xph = []
                pw = COLS // npieces
                for h in range(npieces):
                    xp = xpool.tile([128, pw], _F16, tag=f"x{rc}_{h}")
                    nc.gpsimd.dma_start(xp[:], x[rs, pw * h : pw * (h + 1)])
                    xph.append(xp)
                if rc == 0:
                    nc.sync.dma_start(gt[:], g[:])

                # transpose quads: U[b][t, r] = x[rs, :][r, 128*b + t]
                u_quads = {}
                for tq in range(NQ):
                    ptt = ptpool.tile([128, 512], _F16, tag="pt")
                    for j in range(4):
                        b = 4 * tq + j
                        xp = xph[(128 * b) // pw]
                        bb = b - (128 * b) // pw * (pw // 128)
                        nc.tensor.transpose(
                            ptt[:, 128 * j : 128 * (j + 1)],
                            xp[:, 128 * bb : 128 * (bb + 1)],
                            idt[:],
                        )
                    uq = upool.tile([128, 512], _F16, tag=f"u{tq}")
                    # bitcast fp16 pairs to uint32: halves the column count
                    # (copies are column-rate-bound). DVE only — the ACT
                    # activation path mangles raw uint32 bit patterns.
                    nc.vector.tensor_copy(
                        uq[:].bitcast(mybir.dt.uint32),
                        ptt[:].bitcast(mybir.dt.uint32),
                    )
                    u_quads[tq] = uq

                def u_slice(b, u_quads=u_quads):
                    return u_quads[b // 4][:, 128 * (b % 4) : 128 * (b % 4 + 1)]

                for pg in range(2):  # output panels of 2048 cols (4 banks)
                    gp = rc * 2 + pg
                    yp = ypool.tile([128, 2048], _F16, tag="y")
                    for qq in range(4):
                        q = 4 * pg + qq
                        pt_w = popool.tile([128, 512], _F32, tag="po")
                        groups = [(0, 4, pt_w)]
                        # deep-history (d=1) matmuls first: the opener zeroes
                        # the whole bank (start=True); later d=1 writes land on
                        # pending-zero columns (overwrite), then d=0 writes
                        # accumulate — every instruction's span is uniformly
                        # fresh-or-accumulating.
                        for j0, j1, pt in groups:
                            nmm = sum(1 for dd in range(D) for j in range(j0, j1)
                                      if 4 * q + j - dd >= 0)
                            k = 0
                            for dd in range(D - 1, -1, -1):
                                for j in range(j0, j1):
                                    b = 4 * q + j - dd
                                    if b < 0:
                                        continue
                                    k += 1
                                    nc.tensor.matmul(
                                        pt[:, 128 * (j - j0) : 128 * (j - j0 + 1)],
                                        lhsT=u_slice(b),
                                        rhs=gt[:, 128 * dd : 128 * (dd + 1)],
                                        start=(k == 1),
                                        stop=(k == nmm),
                                    )
                        # cast-copy f32 PSUM -> fp16 SBUF; mostly ACT (DVE
                        # already carries the uint32 u-copies)
                        ceng = (
                            nc.vector.tensor_copy if q % 4 == 0 else nc.scalar.copy
                        )
                        ceng(yp[:, 512 * qq : 512 * (qq + 1)], groups[0][2][:])
                    # stores on SP (loads own Pool); split the last chunk's
                    # panels across SP+Pool so the tail stores parallelize
                    c0 = 2048 * pg
                    if gp == 7:
                        nc.sync.dma_start(y[rs, c0 : c0 + 1024], yp[:, 0:1024])
                        nc.gpsimd.dma_start(
                            y[rs, c0 + 1024 : c0 + 1536], yp[:, 1024:1536]
                        )
                        nc.sync.dma_start(
                            y[rs, c0 + 1536 : c0 + 1792], yp[:, 1536:1792]
                        )
                        nc.gpsimd.dma_start(
                            y[rs, c0 + 1792 : c0 + 2048], yp[:, 1792:2048]
                        )
                    elif gp == 6:
                        nc.sync.dma_start(y[rs, c0 : c0 + 1024], yp[:, 0:1024])
                        nc.gpsimd.dma_start(
                            y[rs, c0 + 1024 : c0 + 2048], yp[:, 1024:2048]
                        )
                    else:
                        nc.sync.dma_start(y[rs, c0 : c0 + 2048], yp[:])
    if legalize:
        _legalize_waits(nc)
    return nc


def _legalize_waits(nc: bass.Bass) -> None:
    """This toolchain's walrus accepts at most ONE semaphore wait per
    instruction (Drain/EventSemaphore excepted), but Tile's semaphore
    assignment freely emits 2-3. Hoist extra waits onto injected same-engine
    NoOps placed immediately before the instruction — engines execute their
    stream serially (and a DMA trigger precedes its descriptor execution),
    so waiting earlier on the same engine preserves semantics.
    """
    for fn in nc.m.functions:
        for blk in fn.blocks:
            out = []
            changed = False
            for i in blk.instructions:
                tn = type(i).__name__
                si = i.sync_info
                cap = 2 if tn == "InstEventSemaphore" else 1
                if si is not None and len(si.on_wait) > cap:
                    waits = list(si.on_wait)
                    for w in waits[:-cap]:
                        out.append(
                            mybir.InstNoOp(
                                name=nc.get_next_instruction_name(),
                                ins=[],
                                outs=[],
                                engine=i.engine,
                                sync_info=mybir.SyncInfo(
                                    on_wait=[w], on_update=[]
                                ),
                            )
                        )
                    i.sync_info = mybir.SyncInfo(
                        on_wait=waits[-cap:], on_update=list(si.on_update)
                    )
                    changed = True
                out.append(i)
            if changed:
                blk.instructions = out


_PROGRAM = None


def kernel(**inputs: np.ndarray) -> np.ndarray:
    global _PROGRAM
    x = np.asarray(inputs["inputs"], dtype=np.float32)
    h = np.asarray(inputs["kernel"], dtype=np.float32)[0]
    assert x.shape == (ROWS, COLS) and h.shape == (K_TAPS,)

    g_cat = _build_g_cat(h)
    xm = x.astype(np.float16)
    xm[:, :K_TAPS] = 0

    if _PROGRAM is None:
        _PROGRAM = _build_program()

    ident = np.eye(128, dtype=np.float16)
    in_maps = [
        {
            "x": xm[ROWS_PER_CORE * c : ROWS_PER_CORE * (c + 1)],
            "g": g_cat,
            "ident": ident,
        }
        for c in range(N_CORES)
    ]
    # the axon-proxied device occasionally reports a transient
    # NRT_EXEC_UNIT_UNRECOVERABLE; a retry succeeds
    last_err = None
    for _ in range(3):
        try:
            res = run_bass_kernel_spmd(
                _PROGRAM, in_maps, list(range(N_CORES))
            ).results
            break
        except Exception as e:  # noqa: BLE001
            last_err = e
            time.sleep(2.0)
    else:
        raise last_err
    out = np.concatenate([res[c]["y"] for c in range(N_CORES)], axis=0)
    return out.astype(np.float32)
